# revision 9
# baseline (speedup 1.0000x reference)
"""KV-cache ring-buffer update + rolling re-linearization, on 8 trn2 NeuronCores.

Problem semantics (nn_KVCache):
  k_cache[:, pos] = k ; v_cache[:, pos] = v          (scatter into ring buffer)
  out = concat([cache[:, split:], cache[:, :split]]) (roll to logical order)

For the given inputs (pos = arange(7680..8703) % 8192, max_pos = 8703) the
whole computation reduces to contiguous row copies:
  out[:, 0:7168]    = cache[:, 512:7680]   (old data, 28 MiB per tensor/batch)
  out[:, 7168:8192] = new[:, 0:1024]       ( 4 MiB per tensor/batch)

Sharding: pure batch-parallel (B=8 -> 1 batch per core), no communication.

Per core this is 64 MiB of DRAM->DRAM copy: 4 InstDMACopy on the sync
HWDGE ring (qSPDynamicHW), which splits each into 64 KiB descriptors
spread round-robin over the 16 SDMA engines (28+4 descriptors per engine
per tensor — perfectly balanced), one completion semaphore, one wait.

Measured on HW (bench.py, in-NEFF loop differencing on the axon cores):
 - solo-core DMA window: ~148-227 us for the 64 MiB (up to ~900 GB/s of
   HBM traffic), varying with terminal load — at/near the HBM path limit.
 - adjacent core pairs (2i, 2i+1) share that path: two active pair-mates
   halve each other; 8-core steady state is ~437 us/round. The harness's
   single-shot number lands near the solo window because the per-core
   NEFF executions are dispatched with large skew.
 - structure variants (second HWDGE ring for v, 4-way split, issue-order
   shuffles, bigger descriptor dtypes) do not beat this: the window is
   bandwidth-bound and the emitted descriptors are already 64 KiB
   (walrus normalizes), so only fixed overheads were left to trim.

Fixed-overhead trims (both verified bit-exact on HW):
 - const-AP MEMSETs stripped (match .memref — .name matched nothing on
   this bass): nothing reads them, and their early start otherwise pins
   first_useful_time ~1.3 us before the first real instruction.
 - the framework's 5-engine start barrier + idle-engine register init
   stripped (_strip_preamble): semaphores are runtime-zeroed per exec and
   rings are quiescent at exec start, so it only delayed the first DMA
   doorbell by the cross-engine handshake.
 - each DMA incs one semaphore (16x per ring split); SP waits >= 16*n_dma
   before retiring — the program must not end before the DMAs land (the
   NEFF postamble does NOT drain HWDGE rings).
"""

import numpy as np

B, S_NEW, H, D = 8, 1024, 16, 128
MAX_SIZE = 8192
HD = H * D  # 2048 fp16 elements = 4096 B per row

N_CORES = 8


def _copy_plan(pos, max_pos):
    """Derive the list of contiguous row-copies implied by (pos, max_pos).

    Returns (out_rows, runs) with runs = [(dst_row, src: 'new'|'cache',
    src_row, n_rows), ...] such that
      out[dst:dst+n] = (k|v)[src_row:src_row+n]        if src == 'new'
      out[dst:dst+n] = (k|v)_cache[src_row:src_row+n]  if src == 'cache'
    """
    pos = (np.asarray(pos).astype(np.int64) % MAX_SIZE).ravel()
    next_pos = int(max_pos) + 1
    if next_pos > MAX_SIZE:
        out_rows = MAX_SIZE
        split = next_pos % MAX_SIZE
        order = (np.arange(MAX_SIZE, dtype=np.int64) + split) % MAX_SIZE
    else:
        out_rows = next_pos
        order = np.arange(next_pos, dtype=np.int64)
    newpos = np.full(MAX_SIZE, -1, dtype=np.int64)
    newpos[pos] = np.arange(pos.shape[0], dtype=np.int64)  # duplicate pos: last wins
    sel = newpos[order]
    is_new = sel >= 0
    src_row = np.where(is_new, sel, order)
    runs = []
    j = 0
    while j < out_rows:
        s = j
        while (
            j + 1 < out_rows
            and is_new[j + 1] == is_new[s]
            and src_row[j + 1] == src_row[s] + (j + 1 - s)
        ):
            j += 1
        runs.append((s, "new" if is_new[s] else "cache", int(src_row[s]), j - s + 1))
        j += 1
    return out_rows, runs


def _strip_const_memsets(nc):
    """Drop the framework's const-AP MEMSETs: nothing in this kernel reads
    them, and their start timestamp otherwise pins first_useful_time.
    The memset's PhysicalAccessPattern carries the tensor name in .memref
    (older bass exposed .name) — match both."""
    def _is_const_memset(i):
        if type(i).__name__ != "InstMemset":
            return False
        return any(
            str(getattr(o, "memref", "") or getattr(o, "name", "")).startswith(
                "const-"
            )
            for o in (i.outs or [])
        )

    for func in nc.m.functions:
        for blk in func.blocks:
            blk.instructions = [
                i for i in blk.instructions if not _is_const_memset(i)
            ]


def _strip_preamble(nc):
    """Remove the framework's 5-engine start barrier (per-engine Drain +
    EventSemaphore handshake through Pool) and ALL register-init MOVEs,
    leaving SP: 4 DMAs -> final wait (7 instructions total incl. ordering
    mode + label). The runtime zeroes semaphores before each execution and
    all DMA rings are quiescent at single-exec start, so the barrier guards
    nothing here; the MOVEs are dead code for this program (DMA patterns and
    offsets are immediate-encoded, shape_reg unused). Removing both takes
    the handshake + 5 MOVEs off the critical path to the first DMA doorbell.
    Verified bit-exact on hardware (bench.py 'nobarrier' and 'leanest').
    Idle engines (Pool/PE/Act/DVE) reduce to 2-instruction stubs.

    Gated: strips only when the pre-DMA region contains exactly the known
    framework preamble instruction types (Call/RegisterMove/Drain/
    EventSemaphore). If a different bass ever emits something unexpected
    there, we keep the full (correct, ~2 us slower) program instead of
    risking correctness for the trim."""
    expected_pre = {"InstCall", "InstRegisterMove", "InstDrain", "InstEventSemaphore"}
    for func in nc.m.functions:
        for blk in func.blocks:
            pre = []
            for i in blk.instructions:
                if type(i).__name__ == "InstDMACopy":
                    break
                pre.append(type(i).__name__)
            if not set(pre) <= expected_pre:
                return  # unexpected preamble shape: leave the program intact
    for func in nc.m.functions:
        for blk in func.blocks:
            out = []
            seen_dma = False
            for i in blk.instructions:
                tn = type(i).__name__
                if tn == "InstDMACopy":
                    seen_dma = True
                if tn in ("InstDrain", "InstRegisterMove"):
                    continue
                if tn == "InstEventSemaphore" and not seen_dma:
                    continue
                out.append(i)
            blk.instructions = out


def _build(out_rows, runs):
    import concourse.bass as bass
    import concourse.mybir as mybir

    nc = bass.Bass(
        enable_partition_id=False,
        monotonic_sem_count=0,
        detect_race_conditions=False,
    )
    f16 = mybir.dt.float16
    kc = nc.declare_dram_parameter("kc", [MAX_SIZE, HD], f16, isOutput=False)
    vc = nc.declare_dram_parameter("vc", [MAX_SIZE, HD], f16, isOutput=False)
    kn = nc.declare_dram_parameter("kn", [S_NEW, HD], f16, isOutput=False)
    vn = nc.declare_dram_parameter("vn", [S_NEW, HD], f16, isOutput=False)
    ko = nc.declare_dram_parameter("ko", [out_rows, HD], f16, isOutput=True)
    vo = nc.declare_dram_parameter("vo", [out_rows, HD], f16, isOutput=True)

    # Single ring (sync HWDGE): its 16 queue-lets feed all 16 SDMA engines by
    # themselves, so a second ring adds no bandwidth (ABBA-paired duel:
    # equal window minima, tworing median slightly worse) and a third queue
    # via gpsimd SWDGE measured ~7 us/round slower. One ring, one
    # semaphore, one waiter.
    # Every dynamic DMA needs a completion semaphore (walrus rejects the
    # program otherwise); all increment the same one, one engine waits.
    sem = nc.alloc_semaphore("sem")
    n_dma = 0
    for dst, src, row, n in runs:
        sk = kn if src == "new" else kc
        sv = vn if src == "new" else vc
        nc.sync.dma_start(out=ko[dst : dst + n], in_=sk[row : row + n]).then_inc(
            sem, 16
        )
        nc.sync.dma_start(out=vo[dst : dst + n], in_=sv[row : row + n]).then_inc(
            sem, 16
        )
        n_dma += 2
    nc.sync.wait_ge(sem, 16 * n_dma)

    _strip_const_memsets(nc)
    _strip_preamble(nc)
    return nc


def _run(k, v, k_cache, v_cache, pos, max_pos, trace=False):
    from concourse.bass_utils import run_bass_kernel_spmd

    k = np.asarray(k)
    v = np.asarray(v)
    k_cache = np.asarray(k_cache)
    v_cache = np.asarray(v_cache)

    out_rows, runs = _copy_plan(pos, max_pos)
    nc = _build(out_rows, runs)

    in_maps = [
        {
            "kc": k_cache[b].reshape(MAX_SIZE, HD),
            "vc": v_cache[b].reshape(MAX_SIZE, HD),
            "kn": k[b].reshape(S_NEW, HD),
            "vn": v[b].reshape(S_NEW, HD),
        }
        for b in range(N_CORES)
    ]
    res = run_bass_kernel_spmd(nc, in_maps, list(range(N_CORES)), trace=trace)
    k_out = np.stack([r["ko"] for r in res.results]).reshape(B, out_rows, H, D)
    v_out = np.stack([r["vo"] for r in res.results]).reshape(B, out_rows, H, D)
    return (k_out, v_out), res


def kernel(k, v, k_cache, v_cache, pos, max_pos):
    (k_out, v_out), _ = _run(k, v, k_cache, v_cache, pos, max_pos)
    return k_out, v_out

